# revision 1
# baseline (speedup 1.0000x reference)
"""AffinityEnergyLoss on 8 Trainium2 NeuronCores (Bass/Tile).

Sharding: core k handles (layer l = k // 4, batch b = k % 4) — one
(l, b) slab of the encoder attns (8 heads x 1025 x 1025, CLS row/col
cropped) plus the matching slab of decoder attns (8 x 1024 x 1024),
~67 MB per core. The kernel is HBM-bandwidth bound: the bulk stream
runs at the per-NC HBM cap (~365 GB/s), so the wall-clock levers are
startup latency, the straggler DMA engine, and the compute tail.

Bulk stream: SWDGE cast-DMA (fp32 -> bf16 in flight) with TWO
consecutive map rows packed per SBUF partition, so each DMA
descriptor covers 8.2 KB of contiguous HBM (for enc maps the full
1025-wide rows INCLUDING the CLS column are read and the CLS element
is skipped in compute — that's what keeps the row pair contiguous).
Halving the descriptor count halves the SWDGE descriptor-ring fetch
traffic whose SBUF-AXI-port contention makes SDMA engine 15 ~13%
slower than its peers at 4 KB packets (the stream is partition-pinned,
so the whole kernel waits for that one engine). HWDGE would avoid the
ring entirely but its RTL descriptor generation blocks the issuing
sequencer ~1.4 us per dma_start and caps engine occupancy at ~76%.

Blocks are 256 map rows: partition p holds rows (2p, 2p+1), giving
even/odd PSUM accumulators. Per map M:
    rowsums s  (one DVE reduce over the [128, 2, 1024] row-pair view,
                bf16 out — halves the DVE write path; the 0.4% bf16
                rowsum quantization averages out across the 32 maps)
    r = 1/s    (DVE reciprocal, [128, 2])
    S_eo += diag(r_eo) @ M_eo  (PE bf16 matmuls into f32 PSUM over all
                                16 maps; diag built on ACT as eye*r
                                via the per-partition activation scale)
Then per block: T = S^T (DVE 32x32 stream transpose, SBUF->SBUF —
keeps the transposes and PSUM round-trip copies off the saturated
PE), Z^T = Pa^T @ T (bf16 PE matmul; Pa = softmax(preds_b) built
on-device). Engine balance per 256-row block against the ~42 us DMA
pace: DVE ~44 us (reduces dominate), PE ~44 us (incl. keep-warm tax),
ACT ~20 us. Tiny bf16 keep-warm matmuls every other map hold the PE
HAM clock gate up — without them MATMUL issue latency roughly
doubles (310 ns -> 595 ns for a [128,512] bf16 matmul).

Host: de-interleave even/odd rows, affinity_raw_b = (Z_l0 + Z_l1)/32,
row-normalize, loss = sum(roi * |softmax(preds) - affinity|) / N.

bf16 map quantization (~0.1% element rms) averages out across the 32
maps and 1024-term dot products: measured rel err ~1e-6 vs the fp32
reference, far under the 2e-2 gate.
"""
import numpy as np

import concourse.bacc as bacc
import concourse.mybir as mybir
import concourse.tile as tile
from concourse.bass_utils import run_bass_kernel_spmd

F32 = mybir.dt.float32
BF16 = mybir.dt.bfloat16
AX = mybir.AxisListType.X
ACTF = mybir.ActivationFunctionType

HEADS = 8
TOK = 1024
C = 21
PB = 128           # partitions
BROWS = 2 * PB     # map rows per block (2 rows per partition)
NB2 = TOK // BROWS  # 4 blocks
NCH = TOK // PB    # 8 column chunks for transpose/Z

_NC = None


def _build_nc():
    nc = bacc.Bacc(None, target_bir_lowering=False)
    enc = nc.dram_tensor("enc", [HEADS, 1025, 1025], F32, kind="ExternalInput")
    dec = nc.dram_tensor("dec", [HEADS, TOK, TOK], F32, kind="ExternalInput")
    pt = nc.dram_tensor("pt", [TOK, C], F32, kind="ExternalInput")
    eye = nc.dram_tensor("eye", [PB, PB], F32, kind="ExternalInput")
    z = nc.dram_tensor("z", [NB2, C, 2, PB], F32, kind="ExternalOutput")

    def _pair_src(ib, m, nh):
        # heads m..m+nh of the block's 16 maps (0-7 enc, 8-15 dec),
        # two consecutive rows per partition
        i0 = ib * BROWS
        if m < 8:
            s = enc[m : m + nh, 1 + i0 : 1 + i0 + BROWS, :]
        else:
            s = dec[m - 8 : m - 8 + nh, i0 : i0 + BROWS, :]
        return s.rearrange("h (p two) c -> p h two c", two=2)

    def _chunk_plan(ib):
        # (start_map, n_heads) per DMA chunk for block ib
        if ib == 0:
            # small leading chunks so the first bytes land ASAP
            return [(0, 1), (1, 1), (2, 2), (4, 2), (6, 2),
                    (8, 2), (10, 2), (12, 2), (14, 2)]
        if ib == NB2 - 1:
            # small trailing chunks to shrink the serial tail
            return [(0, 2), (2, 2), (4, 2), (6, 2), (8, 2),
                    (10, 2), (12, 2), (14, 1), (15, 1)]
        return [(0, 2), (2, 2), (4, 2), (6, 2),
                (8, 2), (10, 2), (12, 2), (14, 2)]

    def _tile_for(m):
        # enc rows are 1025 wide (CLS col included), dec rows 1024
        return ([PB, 2, 2, 1025], "chunkE") if m < 8 else ([PB, 2, 2, 1024], "chunkD")

    with tile.TileContext(nc) as tc:
        with (
            tc.tile_pool(name="const", bufs=1) as const,
            tc.tile_pool(name="stats", bufs=8) as stats,
            tc.tile_pool(name="big", bufs=6) as big,
            tc.tile_pool(name="spool", bufs=2) as spool,
            tc.tile_pool(name="zout", bufs=2) as zout,
            tc.tile_pool(name="psSe", bufs=1, space="PSUM") as psSe,
            tc.tile_pool(name="psSo", bufs=1, space="PSUM") as psSo,
            tc.tile_pool(name="psZe", bufs=1, space="PSUM") as psZe,
            tc.tile_pool(name="psZo", bufs=1, space="PSUM") as psZo,
            tc.tile_pool(name="psW", bufs=1, space="PSUM") as psW,
        ):
            # issue the first block's loads before anything else
            chunk_tiles = {}
            for ci, (m0, nh) in enumerate(_chunk_plan(0)):
                shape, tag = _tile_for(m0)
                t = big.tile(shape, BF16, tag=tag, name=f"pre{ci}")
                nc.gpsimd.dma_start(
                    out=t[:, 0:nh, :, :], in_=_pair_src(0, m0, nh)
                )
                chunk_tiles[(0, ci)] = t

            eye_sb = const.tile([PB, PB], F32)
            nc.sync.dma_start(out=eye_sb[:], in_=eye[:])
            eye_bf = const.tile([PB, PB], BF16)
            nc.scalar.copy(out=eye_bf[:], in_=eye_sb[:])

            # keep-warm: tiny bf16 matmuls interleaved with the real stream
            # keep the PE HAM activity monitor busy enough to hold K=8/8
            # (without them MATMUL issue latency roughly doubles).
            wu_a = const.tile([PB, 64], BF16)
            nc.vector.memset(wu_a[:], 0.0)
            wu_ps = psW.tile([PB, 64], F32)
            wu_n = [0]

            def _warm(k=1):
                for _ in range(k):
                    nc.tensor.matmul(
                        wu_ps[0:64, :], wu_a[:, 0:64], wu_a[:],
                        start=(wu_n[0] == 0), stop=False,
                    )
                    wu_n[0] += 1

            pt_sb = const.tile([PB, NCH, C], F32)
            nc.sync.dma_start(
                out=pt_sb[:], in_=pt.rearrange("(c p) n -> p c n", p=PB)
            )
            pa_sb = const.tile([PB, NCH, C], BF16)
            for c in range(NCH):
                negmx = stats.tile([PB, 1], F32, tag="negmx")
                nc.vector.reduce_max(negmx[:], pt_sb[:, c, :], axis=AX, negate=True)
                ssum = stats.tile([PB, 1], F32, tag="ssum")
                ex = stats.tile([PB, C], F32, tag="ex")
                nc.scalar.activation(
                    ex[:],
                    pt_sb[:, c, :],
                    ACTF.Exp,
                    bias=negmx[:],
                    accum_out=ssum[:],
                )
                rs = stats.tile([PB, 1], F32, tag="rs")
                nc.vector.reciprocal(rs[:], ssum[:])
                nc.vector.tensor_scalar_mul(pa_sb[:, c, :], ex[:], rs[:])

            for ib in range(NB2):
                Se = psSe.tile([PB, TOK], F32)
                So = psSo.tile([PB, TOK], F32)
                for ci, (m0, nh) in enumerate(_chunk_plan(ib)):
                    t = chunk_tiles.pop((ib, ci), None)
                    if t is None:
                        shape, tag = _tile_for(m0)
                        t = big.tile(shape, BF16, tag=tag)
                        nc.gpsimd.dma_start(
                            out=t[:, 0:nh, :, :], in_=_pair_src(ib, m0, nh)
                        )
                    for hm in range(nh):
                        m = m0 + hm
                        off = 1 if m < 8 else 0  # skip CLS col on enc maps
                        me = t[:, hm, 0, off : off + TOK]
                        mo = t[:, hm, 1, off : off + TOK]
                        # rowsum of both rows in one DVE reduce; bf16 out
                        # halves the DVE write path
                        s2 = stats.tile([PB, 2], BF16, tag="s2")
                        with nc.allow_low_precision(
                            reason="rowsum ~512 in bf16: 0.4% worst-case, "
                            "averages out across 32 maps"
                        ):
                            nc.vector.reduce_sum(
                                s2[:], t[:, hm, :, off : off + TOK], axis=AX
                            )
                        r2 = stats.tile([PB, 2], F32, tag="r2")
                        nc.vector.reciprocal(r2[:], s2[:])
                        # diag builds on ACT (eye * r with per-partition scale)
                        dg_e = stats.tile([PB, PB], BF16, tag="dg_e")
                        nc.scalar.activation(
                            dg_e[:], eye_bf[:], ACTF.Copy, scale=r2[:, 0:1]
                        )
                        dg_o = stats.tile([PB, PB], BF16, tag="dg_o")
                        nc.scalar.activation(
                            dg_o[:], eye_bf[:], ACTF.Copy, scale=r2[:, 1:2]
                        )
                        st = (m == 0)
                        sp = (m == 15)
                        nc.tensor.matmul(
                            Se[:, 0:512], dg_e[:], me[:, 0:512], start=st, stop=sp
                        )
                        nc.tensor.matmul(
                            Se[:, 512:1024], dg_e[:], me[:, 512:1024],
                            start=st, stop=sp,
                        )
                        nc.tensor.matmul(
                            So[:, 0:512], dg_o[:], mo[:, 0:512], start=st, stop=sp
                        )
                        nc.tensor.matmul(
                            So[:, 512:1024], dg_o[:], mo[:, 512:1024],
                            start=st, stop=sp,
                        )
                        if m % 2 == 0:
                            _warm(1)

                Se_sb = spool.tile([PB, TOK], BF16, tag="Se")
                nc.scalar.copy(out=Se_sb[:], in_=Se[:])
                So_sb = spool.tile([PB, TOK], BF16, tag="So")
                nc.scalar.copy(out=So_sb[:], in_=So[:])
                z_sb = zout.tile([C, 2, PB], F32)
                for eo, (S_sb, psZ) in enumerate(((Se_sb, psZe), (So_sb, psZo))):
                    T_sb = spool.tile([PB, NCH, PB], BF16, tag=f"T{eo}")
                    for jc in range(NCH):
                        # DVE 32x32-block transpose, SBUF->SBUF: keeps the
                        # transposes (and the PSUM round-trip copies) off PE
                        nc.vector.transpose(
                            T_sb[:, jc, :], S_sb[:, jc * PB : (jc + 1) * PB]
                        )
                    z_ps = psZ.tile([C, PB], F32)
                    for jc in range(NCH):
                        nc.tensor.matmul(
                            z_ps[:], pa_sb[:, jc, :], T_sb[:, jc, :],
                            start=(jc == 0), stop=(jc == NCH - 1),
                        )
                    nc.vector.tensor_copy(z_sb[:, eo, :], z_ps[:])
                nc.sync.dma_start(out=z[ib, :, :, :], in_=z_sb[:])

    nc.compile()
    return nc


def _get_nc():
    global _NC
    if _NC is None:
        _NC = _build_nc()
    return _NC


def kernel(preds, low_feats, high_feats, unlabeled_ROIs, targets, attns, decode_attns):
    preds = np.asarray(preds, dtype=np.float32)
    attns = np.asarray(attns, dtype=np.float32)
    decode_attns = np.asarray(decode_attns, dtype=np.float32)
    roi = np.asarray(unlabeled_ROIs)

    bz = preds.shape[0]
    preds_t = np.ascontiguousarray(
        preds.reshape(bz, C, TOK).transpose(0, 2, 1)
    )  # (bz, 1024, 21)
    eye_np = np.eye(PB, dtype=np.float32)

    nc = _get_nc()
    in_maps = []
    for k in range(8):
        l, b = k // 4, k % 4
        in_maps.append(
            {
                "enc": np.ascontiguousarray(attns[l, b]),
                "dec": np.ascontiguousarray(decode_attns[l, b]),
                "pt": preds_t[b],
                "eye": eye_np,
            }
        )
    res = run_bass_kernel_spmd(nc, in_maps, core_ids=list(range(8)))
    # z per core: (NB2, C, 2, PB) holding Z^T per block, even/odd rows
    # interleaved: token = ib*256 + 2*p + eo
    zs = np.stack(
        [
            res.results[k]["z"].transpose(0, 3, 2, 1).reshape(TOK, C)
            for k in range(8)
        ]
    )

    # combine: affinity_raw_b = (Z_{l=0,b} + Z_{l=1,b}) / 32
    zb = zs.reshape(2, bz, TOK, C).sum(axis=0) / 32.0
    aff = zb / zb.sum(axis=-1, keepdims=True)

    # host softmax (matches jax.nn.softmax in f32)
    e = np.exp(preds_t - preds_t.max(axis=-1, keepdims=True))
    prob = e / e.sum(axis=-1, keepdims=True)  # (bz, 1024, 21)

    roi_f = roi.astype(np.float32).reshape(bz, TOK, 1)
    n_roi = roi_f.sum()
    loss = (roi_f * np.abs(prob - aff)).sum()
    if n_roi > 0:
        loss = loss / n_roi
    return np.asarray(loss, dtype=np.float32)



# revision 2
# speedup vs baseline: 1.0122x; 1.0122x over previous
"""AffinityEnergyLoss on 8 Trainium2 NeuronCores (Bass/Tile), v4.

Sharding: core k handles (layer l = k // 4, batch b = k % 4): the 8
encoder heads (CLS row/col cropped) + 8 decoder heads of that (l, b)
slab — 16 maps of [1024, 1024].

The kernel is a pure streaming GEMM. The host pre-transposes each
attention map and quantizes it to fp8-e4m3 (quantization error
averages out across the 32 maps and the 1024-term dot products;
measured ~6e-6 rel err on the final loss, gate is 2e-2). Device work
per map:

    W^T[n, i] (+ s[i] in row 21) = [P | 1]^T @ M^T

one PSUM accumulation over the column-chunks (K=128 each, DoubleRow
fp8 processes two chunks per matmul) with the augmented-probability
matrix as stationary operand. The ones column makes the PE produce
the row sums s (needed for the per-map row normalization) for free
in the same matmuls — no DVE reduces, no diag matmuls, no on-chip
transposes. ACT evacuates each map's [22, 1024] PSUM to SBUF and
DMAs it out per map (overlapped).

The per-map division W/s, the cross-map sum, the affinity
renormalization and the final loss are host-side fp32 (tiny:
16 x [22, 1024] per core).

Input stream: 16.8 MB/core of fp8 via HWDGE on the sync queue,
map-pairs packed so each of the 128 partitions reads 16 KB
contiguous per dma; first and last maps split into half-map dmas to
shrink pipeline fill/drain.
"""
import numpy as np
import ml_dtypes

import concourse.bacc as bacc
import concourse.mybir as mybir
import concourse.tile as tile
from concourse.bass_utils import run_bass_kernel_spmd

F32 = mybir.dt.float32
BF16 = mybir.dt.bfloat16
FP8 = mybir.dt.float8e4
NP8 = ml_dtypes.float8_e4m3

# "dr" = DoubleRow, "drsw" = DoubleRowSwInterleave (host-interleaved
# weights, contiguous LDWEIGHTS)
PE_MODE = "dr"

HEADS = 8
TOK = 1024
C = 21
CA = C + 1        # prob columns + ones column (row sums)
CAP = 32          # padded stationary width (dual-fp8 LDWEIGHTS wants
                  # 16B-aligned k-planes; 22-byte stride is not)
PB = 128          # partitions
NJC = TOK // PB   # 8 column chunks (contraction tiles)
NMAP = 2 * HEADS  # 8 enc + 8 dec maps per core
NPAIR = NMAP // 2

_NC = None


def _build_nc():
    perf_mode = {
        "dr": mybir.MatmulPerfMode.DoubleRow,
        "drsw": mybir.MatmulPerfMode.DoubleRowSwInterleave,
    }[PE_MODE]
    nc = bacc.Bacc(None, target_bir_lowering=False)
    # map pairs: partition p holds 16 KB contiguous (2 maps x 8 chunks x 1 KB);
    # 512 B pad per partition-row staggers HBM channel phase across partitions
    maps = nc.dram_tensor("maps", [NPAIR, PB, 2 * NJC * TOK + 512], FP8, kind="ExternalInput")
    # paug: per jc-pair the two k-planes' weights, host-interleaved for
    # drsw ([A31 B31 ... A0 B0], AP [p, col, 2]) or plane-major for dr
    # (AP [p, 2, col])
    pshape = [PB, NJC // 2, 2 * CAP] if PE_MODE == "drsw" else [PB, NJC // 2, 2, CAP]
    paug = nc.dram_tensor("paug", pshape, FP8, kind="ExternalInput")
    z = nc.dram_tensor("z", [NPAIR, CA, 2, TOK], BF16, kind="ExternalOutput")

    with tile.TileContext(nc) as tc:
        with (
            tc.tile_pool(name="const", bufs=1) as const,
            tc.tile_pool(name="spool", bufs=2) as spool,
            tc.tile_pool(name="dpool", bufs=6) as dpool,
            tc.tile_pool(name="wpool", bufs=3) as wpool,
            tc.tile_pool(name="psW", bufs=4, space="PSUM") as psW,
        ):
            pa = const.tile(pshape, FP8)
            nc.scalar.dma_start(out=pa[:], in_=paug[:])

            def _pairsrc(q, mm0, mm1, c0, c1):
                return maps[q, :, : 2 * NJC * TOK].rearrange(
                    "p (m c f) -> p m c f", m=2, c=NJC
                )[:, mm0:mm1, c0:c1]

            def _mms(t, mm, ps, jcs):
                for jc in jcs:
                    st = jc == 0
                    sp = jc == NJC - 2
                    # drsw: flat interleaved [p, 64]; dr: [p, 2, col]
                    if PE_MODE == "drsw":
                        lhsT = pa[:, jc // 2, :]
                    else:
                        lhsT = pa[:, jc // 2, :, :]
                    nc.tensor.matmul(
                        ps[:, 0:512], lhsT, t[:, mm, jc : jc + 2, 0:512],
                        start=st, stop=sp, perf_mode=perf_mode,
                    )
                    nc.tensor.matmul(
                        ps[:, 512:1024], lhsT, t[:, mm, jc : jc + 2, 512:1024],
                        start=st, stop=sp, perf_mode=perf_mode,
                    )

            wcur = [None]

            def _finish_map(ps, m, split=False):
                # evac bf16; ship a pair of maps per out dma (4 KB descs,
                # half the descgens and completion receipts)
                if m % 2 == 0:
                    wcur[0] = wpool.tile([CA, 2, TOK], BF16, tag="w", name=f"w{m}")
                w = wcur[0]
                nc.vector.tensor_copy(w[:, m % 2, :], ps[0:CA, :])
                if m % 2 == 1:
                    nc.scalar.dma_start(out=z[m // 2], in_=w[:])

            # map 0 in two half-map dmas (fast pipeline fill), map 1 single
            t0 = spool.tile([PB, 1, NJC, TOK], FP8, tag="s")
            nc.sync.dma_start(out=t0[:, :, 0:4], in_=_pairsrc(0, 0, 1, 0, 4))
            nc.sync.dma_start(out=t0[:, :, 4:8], in_=_pairsrc(0, 0, 1, 4, 8))
            t1 = spool.tile([PB, 1, NJC, TOK], FP8, tag="s")
            nc.sync.dma_start(out=t1[:], in_=_pairsrc(0, 1, 2, 0, 8))

            ps = psW.tile([CAP, TOK], F32)
            _mms(t0, 0, ps, (0, 2, 4, 6))
            _finish_map(ps, 0)
            ps = psW.tile([CAP, TOK], F32)
            _mms(t1, 0, ps, (0, 2, 4, 6))
            _finish_map(ps, 1)

            for q in range(1, NPAIR):
                last = q == NPAIR - 1
                t = dpool.tile([PB, 2, NJC, TOK], FP8, tag="d")
                if last:
                    # final pair: half-map dmas so the tail only chases
                    # the last 512 KB chunks
                    nc.sync.dma_start(out=t[:, 0:1, 0:4], in_=_pairsrc(q, 0, 1, 0, 4))
                    nc.sync.dma_start(out=t[:, 0:1, 4:8], in_=_pairsrc(q, 0, 1, 4, 8))
                    nc.sync.dma_start(out=t[:, 1:2, 0:4], in_=_pairsrc(q, 1, 2, 0, 4))
                    nc.sync.dma_start(out=t[:, 1:2, 4:8], in_=_pairsrc(q, 1, 2, 4, 8))
                else:
                    nc.sync.dma_start(out=t[:], in_=_pairsrc(q, 0, 2, 0, 8))
                for mm in range(2):
                    m = 2 * q + mm
                    ps = psW.tile([CAP, TOK], F32)
                    _mms(t, mm, ps, (0, 2, 4, 6))
                    _finish_map(ps, m)

    nc.compile()
    return nc


def _get_nc():
    global _NC
    if _NC is None:
        _NC = _build_nc()
    return _NC


def prepare_in_maps(preds, attns, decode_attns):
    """Host-side shard + quantize + transpose into per-core input dicts."""
    preds = np.asarray(preds, dtype=np.float32)
    attns = np.asarray(attns, dtype=np.float32)
    decode_attns = np.asarray(decode_attns, dtype=np.float32)
    bz = preds.shape[0]

    # softmax over classes, fp32, tokens-major: (bz, 1024, 21)
    pt = preds.reshape(bz, C, TOK).transpose(0, 2, 1)
    e = np.exp(pt - pt.max(axis=-1, keepdims=True))
    prob = e / e.sum(axis=-1, keepdims=True)

    paugs = []
    for b in range(bz):
        pl = np.zeros((PB, NJC, CAP), dtype=NP8)
        pl[:, :, C] = 1.0
        # token j = jc*128 + p
        pl[:, :, :C] = (
            prob[b].astype(NP8).reshape(NJC, PB, C).transpose(1, 0, 2)
        )
        if PE_MODE == "drsw":
            # [A31 B31 A30 B30 ... A0 B0] per partition per jc-pair
            pa = np.empty((PB, NJC // 2, CAP, 2), dtype=NP8)
            pa[:, :, :, 0] = pl[:, 0::2, ::-1]
            pa[:, :, :, 1] = pl[:, 1::2, ::-1]
            pa = pa.reshape(PB, NJC // 2, 2 * CAP)
        else:
            pa = np.empty((PB, NJC // 2, 2, CAP), dtype=NP8)
            pa[:, :, 0, :] = pl[:, 0::2, :]
            pa[:, :, 1, :] = pl[:, 1::2, :]
        paugs.append(pa)

    in_maps = []
    for k in range(8):
        l, b = k // 4, k % 4
        # fp8-quantize then lay out M^T as [pair, p, 2, jc, i] with
        # j = jc*128 + p (cheap: the transpose shuffles 1-byte data)
        enc8 = attns[l, b][:, 1:, 1:].astype(NP8)
        dec8 = decode_attns[l, b].astype(NP8)
        m8 = np.empty((NMAP, PB, NJC, TOK), dtype=NP8)
        for h in range(HEADS):
            m8[h] = enc8[h].T.reshape(NJC, PB, TOK).transpose(1, 0, 2)
            m8[HEADS + h] = dec8[h].T.reshape(NJC, PB, TOK).transpose(1, 0, 2)
        pairs = np.zeros((NPAIR, PB, 2 * NJC * TOK + 512), dtype=NP8)
        pairs[:, :, : 2 * NJC * TOK] = (
            m8.reshape(NPAIR, 2, PB, NJC * TOK).transpose(0, 2, 1, 3)
            .reshape(NPAIR, PB, 2 * NJC * TOK)
        )
        in_maps.append({"maps": pairs, "paug": paugs[b]})
    return in_maps, prob


def finish(results, prob, unlabeled_ROIs, bz):
    """Host-side: per-map normalize, combine cores, final loss."""
    aff = np.zeros((bz, TOK, C), dtype=np.float32)
    for k in range(8):
        l, b = k // 4, k % 4
        zc = np.asarray(results[k]["z"]).astype(np.float32)  # (8, 22, 2, 1024)
        zc = zc.transpose(0, 2, 1, 3).reshape(NMAP, CA, TOK)
        w = zc[:, :C]          # (16, 21, 1024)
        s = zc[:, C]           # (16, 1024)
        aff[b] += (w / s[:, None, :]).sum(axis=0).T  # (1024, 21)
    aff /= 2.0 * NMAP
    aff = aff / aff.sum(axis=-1, keepdims=True)

    roi_f = np.asarray(unlabeled_ROIs).astype(np.float32).reshape(bz, TOK, 1)
    n_roi = roi_f.sum()
    loss = (roi_f * np.abs(prob - aff)).sum()
    if n_roi > 0:
        loss = loss / n_roi
    return np.asarray(loss, dtype=np.float32)


def kernel(preds, low_feats, high_feats, unlabeled_ROIs, targets, attns, decode_attns):
    bz = np.asarray(preds).shape[0]
    in_maps, prob = prepare_in_maps(preds, attns, decode_attns)
    nc = _get_nc()
    res = run_bass_kernel_spmd(nc, in_maps, core_ids=list(range(8)))
    return finish(res.results, prob, unlabeled_ROIs, bz)
